# revision 23
# baseline (speedup 1.0000x reference)
"""CapsuleNet forward on 8 Trainium2 NeuronCores (Bass/Tile).

Data-parallel over batch (128 -> 8 x 16). All compute on-device; three tiny
AllReduces provide the cross-core BatchNorm statistics (bn1, bna+bnp, final
logits BN).

Per-core device program:
  conv1 (im2col matmul, K=75) -> BN+relu            [AllReduce #1]
  conv2 (strided-AP matmul, K=1152, custom output-channel order)
      -> BN-a(sigmoid) + BN-poses (group stats)      [AllReduce #2]
  votes: per (x,y) matmul, stationary = poses (K=(i,q)=128, M=(r,b)=64),
      moving = host-built block-diag W (bf16, streamed) -> V fp32 in layout
      [(xy%2, r, b) = 128 partitions, (c, p, xy//2, i) free]
  VB routing (3 iters): moments via per-(c,p) tensor_tensor_reduce,
      quad via fused (V-m)*sqrt(1/Psi) tensor_scalar slices,
      digamma via shifted asymptotic series, softmax over c.
  final logits BN                                    [AllReduce #3]
      -> sigmoid -> (16, 10) per core.

n-index permutation: routing treats n = (xy, i); any bijection is valid
because n is only ever summed over and a_i uses the same indexing.
"""
import numpy as np

DEBUG = False
KKMAX = 9

import concourse.bass as bass
import concourse.tile as tile
from concourse import mybir, bass_isa, library_config
from concourse.vector_clock import ScopedClock

P, D, A, B, C, K, ITER = 4, 16, 128, 32, 10, 6, 3
N = B * K * K
ALPHA0, KAPPA0, NU0 = 1.0, 1.0, float(D + 1)
BN_EPS, EPS = 1e-5, 1e-8
NCORES = 8
BL = 128 // NCORES                 # batch per core
XY = K * K                         # 36
XYH = XY // 2                      # 18
CP = C * P                         # 40
FR = C * XYH * B                   # 5760  (c, xy18, i)
FV = C * P * XYH * B               # 23040 (c, p, xy18, i)
FA = XYH * B                       # 576   (xy18, i)
M1 = 128 * 14 * 14
MA = 128 * XY
MP = 128 * D * XY

f32 = mybir.dt.float32
bf16 = mybir.dt.bfloat16
Alu = mybir.AluOpType
Act = mybir.ActivationFunctionType
Ax = mybir.AxisListType


# ---------------- Tile tail-drain patch ----------------
# This walrus build rejects a Drain carrying >1 sem wait ("Too many sync wait
# commands" in CoreV3 setupSyncWait). Spread the final waits across sync nops.
def _patched_drain(self, tick_clock, wait_clock):
    nc = self.nc
    probe = nc.sync.nop()
    wait_clock.add_sem_waits(probe.ins, ScopedClock({None: tick_clock.global_clock}))
    si = probe.ins.sync_info
    waits = list(si.on_wait or []) if si else []
    if si and waits:
        si.on_wait = waits[:1]
        for w in waits[1:]:
            n2 = nc.sync.nop()
            n2.ins.sync_info = mybir.SyncInfo(on_wait=[w], on_update=[])
    nc.sync.drain()
    nc.all_engine_barrier()
    popped = nc._tile_sem_poison_stack.pop()
    assert popped is self._sem_poison
    nc.clear_and_free_semaphores(list(self.sems.allocated().values()))
    nc.all_engine_barrier()


tile.TileContext._drain_and_barrier = _patched_drain


def _fs(t):
    s = t.shape
    n = 1
    for d in s[1:]:
        n *= d
    return n


def ps_(t, p0, np_, free_dims, foff=0):
    """AP over tile t: partitions [p0, p0+np_), given free dims, free offset."""
    fs = _fs(t)
    return bass.AP(tensor=t.tensor, offset=t.offset + p0 * fs + foff,
                   ap=[[fs, np_]] + [list(d) for d in free_dims])


def dap(t, off, dims):
    return bass.AP(tensor=t.tensor, offset=t.offset + off,
                   ap=[list(d) for d in dims])


def _digamma(nc, pool, x_ap, parts, free, tag):
    """psi(x), x >= 1, ~1e-5 abs err:
    ln(y) - .5/y - 1/(12 y^2) + 1/(120 y^4) - 1/x - 1/(x+1),  y = x+2."""
    sh = [parts, free]
    t0 = pool.tile(sh, f32, tag=tag + "0")
    t1 = pool.tile(sh, f32, tag=tag + "1")
    y = pool.tile(sh, f32, tag=tag + "2")
    ry = pool.tile(sh, f32, tag=tag + "3")
    res = pool.tile(sh, f32, tag=tag + "4")
    nc.vector.reciprocal(out=t0[:], in_=x_ap)
    nc.vector.tensor_scalar_add(y[:], x_ap, 1.0)
    nc.vector.reciprocal(out=t1[:], in_=y[:])
    nc.vector.tensor_scalar_add(y[:], x_ap, 2.0)
    nc.vector.reciprocal(out=ry[:], in_=y[:])
    nc.scalar.activation(out=res[:], in_=y[:], func=Act.Ln)
    nc.vector.tensor_tensor(out=t0[:], in0=t0[:], in1=t1[:], op=Alu.add)
    nc.vector.tensor_tensor(out=res[:], in0=res[:], in1=t0[:], op=Alu.subtract)
    nc.vector.tensor_tensor(out=t1[:], in0=ry[:], in1=ry[:], op=Alu.mult)
    nc.vector.tensor_scalar(out=t1[:], in0=t1[:], scalar1=1.0 / 120.0,
                            scalar2=-1.0 / 12.0, op0=Alu.mult, op1=Alu.add)
    nc.vector.tensor_tensor(out=t1[:], in0=t1[:], in1=ry[:], op=Alu.mult)
    nc.vector.tensor_tensor(out=t1[:], in0=t1[:], in1=ry[:], op=Alu.mult)
    nc.vector.tensor_tensor(out=res[:], in0=res[:], in1=t1[:], op=Alu.add)
    nc.vector.tensor_scalar(out=ry[:], in0=ry[:], scalar1=0.5, scalar2=None,
                            op0=Alu.mult)
    nc.vector.tensor_tensor(out=res[:], in0=res[:], in1=ry[:], op=Alu.subtract)
    return res


def _split_waits(nc, maxw=1):
    """This walrus build rejects >1 sem wait on most instruction structs.
    Hoist extra waits onto same-engine NoOps placed just before the owner
    (engine streams execute block order, so ordering is preserved)."""
    import bass_rust
    k = 0
    for f in nc.m.functions:
        for bb in f.blocks:
            out = []
            for inst in bb.instructions:
                si = inst.sync_info
                if si is not None and si.on_wait and len(si.on_wait) > maxw:
                    waits = list(si.on_wait)
                    si.on_wait = waits[:maxw]
                    for w in waits[maxw:]:
                        k += 1
                        nop = bass_rust.InstNoOp(name=f"WSP-{k}", ins=[], outs=[])
                        nop.engine = inst.engine
                        nop.sync_info = mybir.SyncInfo(on_wait=[w], on_update=[])
                        nc.register_instruction(nop, overwrite=True)
                        out.append(nop)
                out.append(inst)
            bb.instructions = out


def build_nc():
    nc = bass.Bass(target_bir_lowering=False, debug=False, num_devices=NCORES)
    xcol = nc.dram_tensor("xcol", [75, BL * 196], f32, kind="ExternalInput")
    w1t = nc.dram_tensor("w1t", [75, 128], f32, kind="ExternalInput")
    w2t = nc.dram_tensor("w2t", [128, 9, 544], f32, kind="ExternalInput")
    bdw = nc.dram_tensor("bdw", [XY, 128, B * CP], bf16, kind="ExternalInput")
    bn1p = nc.dram_tensor("bn1p", [128, 2], f32, kind="ExternalInput")
    bnab = nc.dram_tensor("bnab", [64, 3], f32, kind="ExternalInput")
    dcb = nc.dram_tensor("dcb", [64, CP], f32, kind="ExternalInput")
    red128 = nc.dram_tensor("red128", [128, 32], f32, kind="ExternalInput")
    red64 = nc.dram_tensor("red64", [128, 64], f32, kind="ExternalInput")
    red16 = nc.dram_tensor("red16", [64, 16], f32, kind="ExternalInput")
    yh = nc.dram_tensor("yh", [BL, C], f32, kind="ExternalOutput")
    dbg = {}
    if DEBUG:
        dbg["h"] = nc.dram_tensor("dbg_h", [128, BL * 196], f32,
                                  kind="ExternalOutput")
        dbg["pcb"] = nc.dram_tensor("dbg_pcb", [128, P * BL * XY], bf16,
                                    kind="ExternalOutput")
        dbg["ai"] = nc.dram_tensor("dbg_ai", [32, BL * XY], f32,
                                   kind="ExternalOutput")
        dbg["pcp"] = nc.dram_tensor("dbg_pcp", [128, P * BL * XY], f32,
                                    kind="ExternalOutput")
        dbg["apre"] = nc.dram_tensor("dbg_apre", [32, BL * XY], f32,
                                     kind="ExternalOutput")
        dbg["ps00"] = nc.dram_tensor("dbg_ps00", [128, 288], f32,
                                     kind="ExternalOutput")
        dbg["at"] = nc.dram_tensor("dbg_at", [128, FA], f32,
                                   kind="ExternalOutput")
        dbg["v"] = nc.dram_tensor("dbg_v", [128, FV], f32,
                                  kind="ExternalOutput")
        dbg["ns"] = nc.dram_tensor("dbg_ns", [64, C + 2 * CP], f32,
                                   kind="ExternalOutput")
        dbg["rq"] = nc.dram_tensor("dbg_rq", [32, FR], f32,
                                   kind="ExternalOutput")
        dbg["psi"] = nc.dram_tensor("dbg_psi", [64, CP], f32,
                                    kind="ExternalOutput")
        dbg["eln"] = nc.dram_tensor("dbg_eln", [16, C], f32,
                                    kind="ExternalOutput")
    groups = [list(range(NCORES))]
    with tile.TileContext(nc) as tc:
        _body(nc, tc, xcol, w1t, w2t, bdw, bn1p, bnab, dcb, red128, red64,
              red16, yh, groups, dbg)
    _split_waits(nc)
    return nc


def _bn_scale_bias(nc, pool, g, gamma_ap, beta_ap, minv, nparts, tag):
    """From allreduced [sum, sumsq] (nparts,2) -> scale/bias (nparts,2) tile.
    minv: AP (nparts,1) of 1/count, or float."""
    sb = pool.tile([nparts, 2], f32, tag=tag + "sb")
    mean = pool.tile([nparts, 1], f32, tag=tag + "mn")
    var = pool.tile([nparts, 1], f32, tag=tag + "vr")
    msq = pool.tile([nparts, 1], f32, tag=tag + "ms")
    eps = pool.tile([nparts, 1], f32, tag=tag + "ep")
    nc.vector.memset(eps[:], BN_EPS)
    if isinstance(minv, float):
        nc.vector.tensor_scalar(out=mean[:], in0=g[:, 0:1], scalar1=minv,
                                scalar2=None, op0=Alu.mult)
        nc.vector.tensor_scalar(out=var[:], in0=g[:, 1:2], scalar1=minv,
                                scalar2=None, op0=Alu.mult)
    else:
        nc.vector.tensor_tensor(out=mean[:], in0=g[:, 0:1], in1=minv, op=Alu.mult)
        nc.vector.tensor_tensor(out=var[:], in0=g[:, 1:2], in1=minv, op=Alu.mult)
    nc.vector.tensor_tensor(out=msq[:], in0=mean[:], in1=mean[:], op=Alu.mult)
    nc.vector.tensor_tensor(out=var[:], in0=var[:], in1=msq[:], op=Alu.subtract)
    nc.scalar.activation(out=var[:], in_=var[:], func=Act.Sqrt, bias=eps[:])
    nc.vector.reciprocal(out=var[:], in_=var[:])       # rstd
    nc.vector.tensor_tensor(out=sb[:, 0:1], in0=gamma_ap, in1=var[:], op=Alu.mult)
    nc.vector.tensor_tensor(out=msq[:], in0=mean[:], in1=sb[:, 0:1], op=Alu.mult)
    nc.vector.tensor_tensor(out=sb[:, 1:2], in0=beta_ap, in1=msq[:], op=Alu.subtract)
    return sb


def _body(nc, tc, xcol, w1t, w2t, bdw, bn1p, bnab, dcb, red128, red64,
          red16, yh, groups, dbg=None):
    dbg = dbg or {}
    import contextlib
    est = contextlib.ExitStack()
    with est:
        dram = est.enter_context(tc.tile_pool(name="dram", bufs=1, space="DRAM"))
        stats = est.enter_context(tc.tile_pool(name="stats", bufs=1))
        always = est.enter_context(tc.tile_pool(name="always", bufs=1))

        At = always.tile([128, XYH, B], f32)       # a_i, routing layout
        r128s = always.tile([128, 32], f32)
        r64s = always.tile([128, 64], f32)
        r16s = always.tile([64, 16], f32)
        nc.sync.dma_start(out=r128s[:], in_=red128[:])
        nc.sync.dma_start(out=r64s[:], in_=red64[:])
        nc.sync.dma_start(out=r16s[:], in_=red16[:])
        pcb = always.tile([128, P, BL, XY], bf16)  # poses bf16

        # ================= stem + caps (pool closed before votes) ==========
        with tc.tile_pool(name="stem", bufs=1) as stem, \
             tc.tile_pool(name="psum1", bufs=2, space="PSUM") as psum1:
            xc = stem.tile([75, BL * 196], f32)
            w1s = stem.tile([75, 128], f32)
            h_pre = stem.tile([128, BL * 196], f32)
            h = stem.tile([128, BL * 196], f32)
            w2s = stem.tile([128, 9, 544], f32)
            pcp = stem.tile([128, P, BL, XY], f32)
            a_pre = stem.tile([32, BL * XY], f32)
            a_i = stem.tile([32, BL * XY], f32)
            scr = stem.tile([128, BL * 196], f32)   # ttr product scratch
            nc.sync.dma_start(out=xc[:], in_=xcol[:])
            nc.sync.dma_start(out=w1s[:], in_=w1t[:])
            nc.sync.dma_start(out=w2s[:], in_=w2t[:])

            # ---- conv1 ----
            NT1 = 448
            for t in range(7):
                ps = psum1.tile([128, NT1], f32, tag="c1")
                nc.tensor.matmul(ps[:], w1s[:], xc[:, t * NT1:(t + 1) * NT1],
                                 start=True, stop=True)
                nc.scalar.activation(out=h_pre[:, t * NT1:(t + 1) * NT1],
                                     in_=ps[:], func=Act.Copy)

            st1 = stats.tile([128, 2], f32)
            nc.vector.reduce_sum(out=st1[:, 0:1], in_=h_pre[:], axis=Ax.X)
            nc.vector.tensor_tensor(out=scr[:], in0=h_pre[:], in1=h_pre[:],
                                    op=Alu.mult)
            nc.vector.reduce_sum(out=st1[:, 1:2], in_=scr[:], axis=Ax.X)

            bc1_i = dram.tile([128, 2], f32)
            bc1_o = dram.tile([128, 2], f32)
            nc.sync.dma_start(out=bc1_i[:], in_=st1[:])
            nc.gpsimd.collective_compute(
                "AllReduce", Alu.add, replica_groups=groups,
                ins=[bc1_i[:].opt()], outs=[bc1_o[:].opt()])
            g1 = stats.tile([128, 2], f32)
            nc.sync.dma_start(out=g1[:], in_=bc1_o[:])
            gb1 = stats.tile([128, 2], f32)
            nc.sync.dma_start(out=gb1[:], in_=bn1p[:])
            sb1 = _bn_scale_bias(nc, stats, g1, gb1[:, 0:1], gb1[:, 1:2],
                                 1.0 / M1, 128, "b1")
            nc.scalar.activation(out=h[:], in_=h_pre[:], func=Act.Relu,
                                 bias=sb1[:, 1:2], scale=sb1[:, 0:1])

            if "h" in dbg:
                nc.sync.dma_start(out=dbg["h"][:], in_=h[:])
            # ---- conv2 ----
            HB = BL // 2
            for m in range(5):
                mp = 128 if m < 4 else 32
                for bh in range(2):
                    ps = psum1.tile([128, HB * XY], f32, tag="c2")
                    for kk in range(KKMAX):
                        kh, kw = kk // 3, kk % 3
                        rhs = dap(h, bh * HB * 196 + kh * 14 + kw,
                                  [[BL * 196, 128], [196, HB], [28, 6], [2, 6]])
                        nc.tensor.matmul(ps[:mp, :],
                                         w2s[:, kk, m * 128:m * 128 + mp],
                                         rhs, start=(kk == 0), stop=(kk == KKMAX - 1))
                    if m == 0 and bh == 0 and "ps00" in dbg:
                        nc.vector.tensor_copy(out=ps_(scr, 0, 128, [[1, 288]]),
                                              in_=ps[:])
                        nc.sync.dma_start(out=dbg["ps00"][:],
                                          in_=ps_(scr, 0, 128, [[1, 288]]))
                    if m < 4:
                        dst = dap(pcp, m * BL * XY + bh * HB * XY,
                                  [[P * BL * XY, 128], [XY, HB], [1, XY]])
                        nc.scalar.activation(out=dst, in_=ps[:], func=Act.Copy)
                    else:
                        nc.scalar.activation(
                            out=a_pre[:, bh * HB * XY:(bh + 1) * HB * XY],
                            in_=ps[:32, :], func=Act.Copy)

            if "pcp" in dbg:
                nc.sync.dma_start(out=dbg["pcp"][:], in_=pcp[:])
            if "apre" in dbg:
                nc.sync.dma_start(out=dbg["apre"][:], in_=a_pre[:])
            # ---- bn-a / bn-poses stats ----
            ast = stats.tile([32, 2], f32)
            pst = stats.tile([128, 2], f32)
            nc.vector.reduce_sum(out=ast[:, 0:1], in_=a_pre[:], axis=Ax.X)
            nc.vector.tensor_tensor(out=ps_(scr, 0, 32, [[1, BL * XY]]),
                                    in0=a_pre[:], in1=a_pre[:], op=Alu.mult)
            nc.vector.reduce_sum(out=ast[:, 1:2],
                                 in_=ps_(scr, 0, 32, [[1, BL * XY]]), axis=Ax.X)
            pcp_f = ps_(pcp, 0, 128, [[1, P * BL * XY]])
            nc.vector.reduce_sum(out=pst[:, 0:1], in_=pcp_f, axis=Ax.X)
            nc.vector.tensor_tensor(out=ps_(scr, 0, 128, [[1, P * BL * XY]]),
                                    in0=pcp_f, in1=pcp_f, op=Alu.mult)
            nc.vector.reduce_sum(out=pst[:, 1:2],
                                 in_=ps_(scr, 0, 128, [[1, P * BL * XY]]),
                                 axis=Ax.X)
            # (i,q) rows -> (q, (i,2)); reduce over q
            qt = stats.tile([4, 64], f32)
            qs = stats.tile([1, 64], f32)
            ones4 = stats.tile([4, 1], f32)
            nc.vector.memset(ones4[:], 1.0)
            pd = dram.tile([256], f32)
            nc.sync.dma_start(
                out=bass.AP(tensor=pd.tensor, offset=pd.offset,
                            ap=[[2, 128], [1, 2]]),
                in_=pst[:])
            nc.sync.dma_start(
                out=dap(qt, 0, [[64, 4], [2, 32], [1, 2]]),
                in_=bass.AP(tensor=pd.tensor, offset=pd.offset,
                            ap=[[64, 4], [2, 32], [1, 2]]))
            pq4 = psum1.tile([1, 64], f32, tag="pq4")
            nc.tensor.matmul(pq4[:], ones4[:], qt[:], start=True, stop=True)
            nc.vector.tensor_copy(out=qs[:], in_=pq4[:])
            bc2_i = dram.tile([64, 2], f32)
            bc2_o = dram.tile([64, 2], f32)
            nc.sync.dma_start(
                out=bass.AP(tensor=bc2_i.tensor, offset=bc2_i.offset,
                            ap=[[2, 32], [1, 2]]), in_=ast[:])
            nc.sync.dma_start(
                out=bass.AP(tensor=bc2_i.tensor, offset=bc2_i.offset + 64,
                            ap=[[2, 32], [1, 2]]),
                in_=dap(qs, 0, [[64, 1], [2, 32], [1, 2]]))  # row 0
            nc.gpsimd.collective_compute(
                "AllReduce", Alu.add, replica_groups=groups,
                ins=[bc2_i[:].opt()], outs=[bc2_o[:].opt()])
            g2 = stats.tile([64, 2], f32)
            nc.sync.dma_start(out=g2[:], in_=bc2_o[:])
            gbab = stats.tile([64, 3], f32)
            nc.sync.dma_start(out=gbab[:], in_=bnab[:])
            sb2 = _bn_scale_bias(nc, stats, g2, gbab[:, 0:1], gbab[:, 1:2],
                                 gbab[:, 2:3], 64, "b2")
            # a = sigmoid(bn(a_pre))
            nc.scalar.activation(out=a_i[:], in_=a_pre[:], func=Act.Sigmoid,
                                 bias=ps_(sb2, 0, 32, [[1, 1]], 1),
                                 scale=ps_(sb2, 0, 32, [[1, 1]], 0))
            # pose scale/bias rows per (i,q) partition
            pssc = stats.tile([128, 1], f32)
            psbi = stats.tile([128, 1], f32)
            sbd = dram.tile([64], f32)
            nc.sync.dma_start(
                out=bass.AP(tensor=sbd.tensor, offset=sbd.offset,
                            ap=[[2, 32], [1, 2]]),
                in_=ps_(sb2, 32, 32, [[1, 2]]))
            for q in range(4):
                nc.sync.dma_start(
                    out=ps_(pssc, q * 32, 32, [[1, 1]]),
                    in_=bass.AP(tensor=sbd.tensor, offset=sbd.offset,
                                ap=[[2, 32], [1, 1]]))
                nc.sync.dma_start(
                    out=ps_(psbi, q * 32, 32, [[1, 1]]),
                    in_=bass.AP(tensor=sbd.tensor, offset=sbd.offset + 1,
                                ap=[[2, 32], [1, 1]]))
            nc.vector.tensor_scalar(out=pcb[:], in0=pcp[:], scalar1=pssc[:],
                                    scalar2=psbi[:], op0=Alu.mult, op1=Alu.add)

            if "pcb" in dbg:
                nc.sync.dma_start(out=dbg["pcb"][:], in_=pcb[:])
            if "ai" in dbg:
                nc.sync.dma_start(out=dbg["ai"][:], in_=a_i[:])
            # a_i -> At via DRAM roundtrip, r-replicated
            a_dr = dram.tile([BL * XY * B], f32)
            nc.sync.dma_start(
                out=bass.AP(tensor=a_dr.tensor, offset=a_dr.offset,
                            ap=[[1, 32], [32, BL * XY]]),
                in_=a_i[:])
            for xy2 in range(2):
                for r in range(4):
                    nc.sync.dma_start(
                        out=ps_(At, xy2 * 64 + r * 16, 16,
                                [[B, XYH], [1, B]]),
                        in_=bass.AP(tensor=a_dr.tensor,
                                    offset=a_dr.offset + xy2 * B,
                                    ap=[[XY * B, 16], [2 * B, XYH], [1, B]]))

        if "at" in dbg:
            nc.sync.dma_start(out=dbg["at"][:], in_=At[:])
        # ================= votes ==========================================
        with tc.tile_pool(name="vpool", bufs=1) as vpool:
            V = vpool.tile([128, C, P, XYH, B], f32)
            with tc.tile_pool(name="bdwc", bufs=2) as bdwc, \
                 tc.tile_pool(name="psumv", bufs=6, space="PSUM") as psumv:
                NI = [12, 12, 8]
                CH = 6                        # xy per streamed chunk
                for ch in range(XY // CH):
                    bt = bdwc.tile([128, CH, B * CP], bf16, tag="bdw")
                    nc.sync.dma_start(out=bt[:], in_=bdw[ch * CH:(ch + 1) * CH]
                                      .rearrange("a b c -> b a c"))
                    for j in range(CH):
                        xy = ch * CH + j
                        lhsT = dap(pcb, xy, [[P * BL * XY, 128],
                                             [BL * XY, P], [XY, BL]])
                        i0 = 0
                        for ni in NI:
                            ps = psumv.tile([64, 480], f32, tag="vv")
                            nc.tensor.matmul(ps[:, :ni * CP], lhsT,
                                             bt[:, j, i0 * CP:(i0 + ni) * CP],
                                             start=True, stop=True)
                            src = dap(ps, 0, [[480, 64], [CP, ni], [P, C], [1, P]])
                            dst = bass.AP(
                                tensor=V.tensor,
                                offset=(V.offset + (xy % 2) * 64 * FV
                                        + (xy // 2) * B + i0),
                                ap=[[FV, 64], [1, ni], [P * XYH * B, C],
                                    [XYH * B, P]])
                            if xy % 2:
                                nc.scalar.activation(out=dst, in_=src, func=Act.Copy)
                            else:
                                nc.vector.tensor_copy(out=dst, in_=src)
                            i0 += ni

            if "v" in dbg:
                nc.sync.dma_start(out=dbg["v"][:], in_=V[:])
            # ================= routing ====================================
            with tc.tile_pool(name="rout", bufs=1) as rp, \
                 tc.tile_pool(name="psumr", bufs=2, space="PSUM") as psumr:
                Ra = rp.tile([128, C, XYH, B], f32)
                Q = rp.tile([128, C, XYH, B], f32)
                scrM = rp.tile([128, C, XYH, B], f32)
                Rq = rp.tile([32, C, XYH, B], f32)    # quad -> lnp -> R
                NS = stats.tile([128, C + 2 * CP], f32)
                NST = stats.tile([64, C + 2 * CP], f32)
                Nj = stats.tile([64, C], f32)
                mm = stats.tile([64, CP], f32)
                srP = stats.tile([64, CP], f32)
                Psi = stats.tile([64, CP], f32)
                m128 = stats.tile([128, CP], f32)
                s128 = stats.tile([128, CP], f32)
                nu32 = stats.tile([32, C], f32)
                rk32 = stats.tile([32, C], f32)
                E32 = stats.tile([32, C], f32)
                dcs = stats.tile([64, CP], f32)
                kap = stats.tile([64, C], f32)
                rn = stats.tile([64, C], f32)
                rkap = stats.tile([64, C], f32)
                nus = stats.tile([64, C], f32)
                t_a = stats.tile([64, CP], f32)
                t_b = stats.tile([64, CP], f32)
                t_c = stats.tile([64, C], f32)
                red = stats.tile([64, C], f32)
                el1 = stats.tile([16, C], f32)
                eln = stats.tile([16, C], f32)
                als = stats.tile([16, 1], f32)
                mx = stats.tile([32, FA], f32)
                nc.sync.dma_start(out=dcs[:], in_=dcb[:])

                at_bc = dap(At, 0, [[FA, 128], [0, C], [B, XYH], [1, B]])

                for it in range(ITER):
                    # ---- Ra ----
                    if it == 0:
                        nc.vector.tensor_scalar(out=Ra[:], in0=at_bc,
                                                scalar1=1.0 / C, scalar2=None,
                                                op0=Alu.mult)
                    else:
                        for xy2 in range(2):
                            for r in range(4):
                                nc.sync.dma_start(
                                    out=ps_(Ra, xy2 * 64 + r * 16, 16, [[1, FR]]),
                                    in_=ps_(Rq, xy2 * 16, 16, [[1, FR]]))
                        nc.vector.tensor_tensor(out=Ra[:], in0=Ra[:], in1=at_bc,
                                                op=Alu.mult)

                    # ---- moments (Nj | S1 | S2 packed in NS cols) ----
                    NSW = C + 2 * CP
                    nc.vector.reduce_sum(out=NS[:, 0:C], in_=Ra[:], axis=Ax.XY)
                    for p4 in range(P):
                        v_p = dap(V, p4 * FA, [[FV, 128], [P * FA, C], [1, FA]])
                        nc.vector.tensor_tensor(out=scrM[:], in0=Ra[:], in1=v_p,
                                                op=Alu.mult)
                        nc.vector.reduce_sum(
                            out=dap(NS, C + p4, [[NSW, 128], [P, C]]),
                            in_=dap(scrM, 0, [[FR, 128], [FA, C], [1, FA]]),
                            axis=Ax.X)
                        nc.vector.tensor_tensor(out=scrM[:], in0=scrM[:], in1=v_p,
                                                op=Alu.mult)
                        nc.vector.reduce_sum(
                            out=dap(NS, C + CP + p4, [[NSW, 128], [P, C]]),
                            in_=dap(scrM, 0, [[FR, 128], [FA, C], [1, FA]]),
                            axis=Ax.X)
                    pns = psumr.tile([64, C + 2 * CP], f32, tag="pns")
                    nc.tensor.matmul(pns[:], r64s[:, 0:64], NS[:],
                                     start=True, stop=True)
                    nc.vector.tensor_copy(out=NST[:], in_=pns[:])
                    nc.vector.tensor_scalar_add(Nj[:], NST[:, 0:C], EPS)
                    if it == 0 and "ns" in dbg:
                        nc.sync.dma_start(out=dbg["ns"][:], in_=NST[:])

                    # ---- posterior stats ----
                    nc.vector.reciprocal(out=rn[:], in_=Nj[:])
                    nc.vector.tensor_scalar_add(kap[:], Nj[:], KAPPA0)
                    nc.vector.reciprocal(out=rkap[:], in_=kap[:])
                    nc.vector.tensor_scalar_add(nus[:], Nj[:], NU0)
                    rn_b = dap(rn, 0, [[C, 64], [1, C], [0, P]])
                    nj_b = dap(Nj, 0, [[C, 64], [1, C], [0, P]])
                    xb = t_a
                    nc.vector.tensor_tensor(
                        out=xb[:], in0=dap(NST, C, [[C + 2 * CP, 64], [1, CP]]),
                        in1=rn_b, op=Alu.mult)
                    nc.vector.tensor_tensor(out=t_c[:], in0=Nj[:], in1=rkap[:],
                                            op=Alu.mult)
                    tc_b = dap(t_c, 0, [[C, 64], [1, C], [0, P]])
                    nc.vector.tensor_tensor(out=mm[:], in0=xb[:], in1=tc_b,
                                            op=Alu.mult)
                    nc.vector.tensor_tensor(out=t_b[:], in0=xb[:], in1=xb[:],
                                            op=Alu.mult)
                    nc.vector.tensor_tensor(out=Psi[:], in0=t_b[:], in1=tc_b,
                                            op=Alu.mult)
                    nc.vector.tensor_tensor(out=t_b[:], in0=t_b[:], in1=nj_b,
                                            op=Alu.mult)
                    nc.vector.tensor_tensor(out=Psi[:], in0=Psi[:], in1=t_b[:],
                                            op=Alu.subtract)
                    nc.vector.tensor_tensor(
                        out=Psi[:], in0=Psi[:],
                        in1=dap(NST, C + CP, [[C + 2 * CP, 64], [1, CP]]),
                        op=Alu.add)
                    nc.vector.tensor_scalar_add(Psi[:], Psi[:], 1.0)
                    if it == 0 and "psi" in dbg:
                        nc.sync.dma_start(out=dbg["psi"][:], in_=Psi[:])

                    # ---- expectations ----
                    alpha = t_c
                    nc.vector.tensor_scalar_add(alpha[:], Nj[:], ALPHA0)
                    dg_a = _digamma(nc, stats, ps_(alpha, 0, 16, [[1, C]]),
                                    16, C, "dga")
                    nc.vector.reduce_sum(out=als[:],
                                         in_=ps_(alpha, 0, 16, [[1, C]]), axis=Ax.X)
                    dg_s = _digamma(nc, stats, als[:], 16, 1, "dgs")
                    nc.vector.tensor_tensor(out=el1[:], in0=dg_a[:],
                                            in1=dap(dg_s, 0, [[1, 16], [0, C]]),
                                            op=Alu.subtract)
                    arg = t_a
                    nc.vector.tensor_scalar(out=arg[:],
                                            in0=dap(nus, 0, [[C, 64], [1, C], [0, P]]),
                                            scalar1=0.5, scalar2=None, op0=Alu.mult)
                    nc.vector.tensor_tensor(out=arg[:], in0=arg[:], in1=dcs[:],
                                            op=Alu.add)
                    dg_n = _digamma(nc, stats, arg[:], 64, CP, "dgn")
                    lnP = t_b
                    nc.scalar.activation(out=lnP[:], in_=Psi[:], func=Act.Ln)
                    nc.vector.tensor_tensor(out=dg_n[:], in0=dg_n[:], in1=lnP[:],
                                            op=Alu.subtract)
                    nc.vector.reduce_sum(out=red[:],
                                         in_=dap(dg_n, 0, [[CP, 64], [P, C], [1, P]]),
                                         axis=Ax.X)
                    pe2 = psumr.tile([16, C], f32, tag="pe2")
                    nc.tensor.matmul(pe2[:], r16s[:], red[:], start=True, stop=True)
                    nc.vector.tensor_copy(out=eln[:], in_=pe2[:])
                    nc.vector.tensor_scalar_add(eln[:], eln[:],
                                                D * float(np.log(2.0)))
                    if it == 0 and "eln" in dbg:
                        nc.sync.dma_start(out=dbg["eln"][:], in_=eln[:])

                    if it == ITER - 1:
                        # logits; final batch BN; sigmoid; out
                        logt = stats.tile([16, C], f32, tag="lg")
                        nc.vector.tensor_scalar(out=logt[:], in0=eln[:],
                                                scalar1=0.5, scalar2=None,
                                                op0=Alu.mult)
                        nc.vector.tensor_tensor(out=logt[:], in0=logt[:],
                                                in1=el1[:], op=Alu.add)
                        pk = stats.tile([16, 2 * C], f32, tag="pk")
                        nc.vector.tensor_copy(out=pk[:, 0:C], in_=logt[:])
                        nc.scalar.activation(out=pk[:, C:2 * C], in_=logt[:],
                                             func=Act.Square)
                        ones16 = stats.tile([16, 1], f32, tag="o16")
                        nc.vector.memset(ones16[:], 1.0)
                        pkr = psumr.tile([1, 2 * C], f32, tag="pkr")
                        nc.tensor.matmul(pkr[:], ones16[:], pk[:],
                                         start=True, stop=True)
                        pks = stats.tile([1, 2 * C], f32, tag="pks")
                        nc.vector.tensor_copy(out=pks[:], in_=pkr[:])
                        bc3_i = dram.tile([1, 2 * C], f32)
                        bc3_o = dram.tile([1, 2 * C], f32)
                        nc.sync.dma_start(out=bc3_i[:], in_=pks[:])
                        nc.gpsimd.collective_compute(
                            "AllReduce", Alu.add, replica_groups=groups,
                            ins=[bc3_i[:].opt()], outs=[bc3_o[:].opt()])
                        g3 = stats.tile([16, 2 * C], f32, tag="g3")
                        nc.sync.dma_start(
                            out=g3[:],
                            in_=bass.AP(tensor=bc3_o.tensor, offset=bc3_o.offset,
                                        ap=[[0, 16], [1, 2 * C]]))
                        mn3 = stats.tile([16, C], f32, tag="m3")
                        vr3 = stats.tile([16, C], f32, tag="v3")
                        ms3 = stats.tile([16, C], f32, tag="s3")
                        ep3 = stats.tile([16, 1], f32, tag="e3")
                        nc.vector.memset(ep3[:], BN_EPS)
                        nc.vector.tensor_scalar(out=mn3[:], in0=g3[:, 0:C],
                                                scalar1=1.0 / 128.0, scalar2=None,
                                                op0=Alu.mult)
                        nc.vector.tensor_scalar(out=vr3[:], in0=g3[:, C:2 * C],
                                                scalar1=1.0 / 128.0, scalar2=None,
                                                op0=Alu.mult)
                        nc.vector.tensor_tensor(out=ms3[:], in0=mn3[:], in1=mn3[:],
                                                op=Alu.mult)
                        nc.vector.tensor_tensor(out=vr3[:], in0=vr3[:], in1=ms3[:],
                                                op=Alu.subtract)
                        nc.scalar.activation(out=vr3[:], in_=vr3[:], func=Act.Sqrt,
                                             bias=ep3[:])
                        nc.vector.reciprocal(out=vr3[:], in_=vr3[:])
                        nc.vector.tensor_tensor(out=logt[:], in0=logt[:], in1=mn3[:],
                                                op=Alu.subtract)
                        nc.vector.tensor_tensor(out=logt[:], in0=logt[:], in1=vr3[:],
                                                op=Alu.mult)
                        yo = stats.tile([16, C], f32, tag="yo")
                        nc.scalar.activation(out=yo[:], in_=logt[:], func=Act.Sigmoid)
                        nc.sync.dma_start(out=yh[:], in_=yo[:])
                        break

                    # ---- quad ----
                    nc.vector.reciprocal(out=srP[:], in_=Psi[:])
                    nc.scalar.activation(out=srP[:], in_=srP[:], func=Act.Sqrt)
                    for s_, d_ in ((mm, m128), (srP, s128)):
                        nc.sync.dma_start(out=ps_(d_, 0, 64, [[1, CP]]), in_=s_[:])
                        nc.sync.dma_start(out=ps_(d_, 64, 64, [[1, CP]]), in_=s_[:])
                    for c in range(C):
                        for p in range(P):
                            nc.vector.tensor_scalar(
                                out=dap(scrM, p * FA, [[FR, 128], [1, FA]]),
                                in0=dap(V, (c * P + p) * FA, [[FV, 128], [1, FA]]),
                                scalar1=dap(m128, c * P + p, [[CP, 128], [1, 1]]),
                                scalar2=dap(s128, c * P + p, [[CP, 128], [1, 1]]),
                                op0=Alu.subtract, op1=Alu.mult)
                        sl = [dap(scrM, p * FA, [[FR, 128], [1, FA]])
                              for p in range(P)]
                        nc.vector.tensor_tensor(out=sl[0], in0=sl[0], in1=sl[0],
                                                op=Alu.mult)
                        for p in range(1, P):
                            nc.vector.tensor_tensor(out=sl[p], in0=sl[p], in1=sl[p],
                                                    op=Alu.mult)
                            dst = (dap(Q, c * FA, [[FR, 128], [1, FA]])
                                   if p == P - 1 else sl[0])
                            nc.vector.tensor_tensor(out=dst, in0=sl[0],
                                                    in1=sl[p], op=Alu.add)
                    # sum over (r) and fold xy2 -> Rq rows (xy2, b), via PE
                    for ck in range(12):
                        pq = psumr.tile([32, 480], f32, tag="pq")
                        nc.tensor.matmul(pq[:], r128s[:],
                                         dap(Q, ck * 480, [[FR, 128], [1, 480]]),
                                         start=True, stop=True)
                        nc.vector.tensor_copy(
                            out=ps_(Rq, 0, 32, [[1, 480]], foff=ck * 480),
                            in_=pq[:])
                    # lnp = E' - .5 (nu*quad + D*rkap)
                    for s_, d_ in ((nus, nu32), (rkap, rk32)):
                        nc.sync.dma_start(out=ps_(d_, 0, 16, [[1, C]]),
                                          in_=ps_(s_, 0, 16, [[1, C]]))
                        nc.sync.dma_start(out=ps_(d_, 16, 16, [[1, C]]),
                                          in_=ps_(s_, 0, 16, [[1, C]]))
                    nc.vector.tensor_scalar(out=eln[:], in0=eln[:], scalar1=0.5,
                                            scalar2=None, op0=Alu.mult)
                    nc.vector.tensor_tensor(out=el1[:], in0=el1[:], in1=eln[:],
                                            op=Alu.add)
                    nc.vector.tensor_scalar_add(
                        el1[:], el1[:], -0.5 * D * float(np.log(2.0 * np.pi)))
                    nc.sync.dma_start(out=ps_(E32, 0, 16, [[1, C]]), in_=el1[:])
                    nc.sync.dma_start(out=ps_(E32, 16, 16, [[1, C]]), in_=el1[:])
                    nc.vector.tensor_tensor(
                        out=Rq[:], in0=Rq[:],
                        in1=dap(nu32, 0, [[C, 32], [1, C], [0, XYH], [0, B]]),
                        op=Alu.mult)
                    nc.vector.tensor_scalar(out=rk32[:], in0=rk32[:],
                                            scalar1=float(D), scalar2=None,
                                            op0=Alu.mult)
                    nc.vector.tensor_tensor(
                        out=Rq[:], in0=Rq[:],
                        in1=dap(rk32, 0, [[C, 32], [1, C], [0, XYH], [0, B]]),
                        op=Alu.add)
                    nc.vector.tensor_scalar(out=Rq[:], in0=Rq[:], scalar1=-0.5,
                                            scalar2=None, op0=Alu.mult)
                    nc.vector.tensor_tensor(
                        out=Rq[:], in0=Rq[:],
                        in1=dap(E32, 0, [[C, 32], [1, C], [0, XYH], [0, B]]),
                        op=Alu.add)
                    # softmax over c (outer free dim)
                    nc.vector.tensor_tensor(out=mx[:],
                                            in0=dap(Rq, 0, [[FR, 32], [1, FA]]),
                                            in1=dap(Rq, FA, [[FR, 32], [1, FA]]),
                                            op=Alu.max)
                    for c in range(2, C):
                        nc.vector.tensor_tensor(
                            out=mx[:], in0=mx[:],
                            in1=dap(Rq, c * FA, [[FR, 32], [1, FA]]), op=Alu.max)
                    nc.vector.tensor_tensor(
                        out=Rq[:], in0=Rq[:],
                        in1=dap(mx, 0, [[FA, 32], [0, C], [1, FA]]),
                        op=Alu.subtract)
                    nc.scalar.activation(out=Rq[:], in_=Rq[:], func=Act.Exp)
                    sm = mx
                    nc.vector.tensor_tensor(out=sm[:],
                                            in0=dap(Rq, 0, [[FR, 32], [1, FA]]),
                                            in1=dap(Rq, FA, [[FR, 32], [1, FA]]),
                                            op=Alu.add)
                    for c in range(2, C):
                        nc.vector.tensor_tensor(
                            out=sm[:], in0=sm[:],
                            in1=dap(Rq, c * FA, [[FR, 32], [1, FA]]), op=Alu.add)
                    nc.vector.reciprocal(out=sm[:], in_=sm[:])
                    nc.vector.tensor_tensor(
                        out=Rq[:], in0=Rq[:],
                        in1=dap(sm, 0, [[FA, 32], [0, C], [1, FA]]), op=Alu.mult)
                    if it == 0 and "rq" in dbg:
                        nc.sync.dma_start(out=dbg["rq"][:], in_=Rq[:])


# =====================================================================
# host side
# =====================================================================
_NC_CACHE = {}


def _get_nc():
    if "nc" not in _NC_CACHE:
        _NC_CACHE["nc"] = build_nc()
    return _NC_CACHE["nc"]


def _prep(conv1_w, prim_w, W_ij, bn1_g, bn1_b, bna_g, bna_b, bnp_g, bnp_b):
    import ml_dtypes
    f = np.float32
    w1t = np.ascontiguousarray(conv1_w.reshape(A, 75).T).astype(f)
    co = np.empty(544, np.int64)
    j = np.arange(128)
    for m in range(4):
        co[m * 128:(m + 1) * 128] = (j % 32) * 16 + (j // 32) * 4 + m
    co[512:544] = 512 + np.arange(32)
    w2t = np.ascontiguousarray(
        prim_w[co].transpose(1, 2, 3, 0).reshape(128, 9, 544)).astype(f)
    bdwv = np.zeros((XY, 128, B * CP), f)
    ii, cc, pp, qq = np.meshgrid(np.arange(B), np.arange(C), np.arange(P),
                                 np.arange(P), indexing="ij")
    for x_ in range(K):
        for y_ in range(K):
            bdwv[x_ * 6 + y_, qq * 32 + ii, ii * CP + cc * 4 + pp] = \
                W_ij[:, :, x_, y_]
    bdwv = bdwv.astype(ml_dtypes.bfloat16)
    bn1p = np.stack([bn1_g, bn1_b], 1).astype(f)
    bnab = np.zeros((64, 3), f)
    bnab[0:32, 0], bnab[0:32, 1], bnab[0:32, 2] = bna_g, bna_b, 1.0 / MA
    bnab[32:64, 0], bnab[32:64, 1], bnab[32:64, 2] = bnp_g, bnp_b, 1.0 / MP
    dcbv = np.zeros((64, CP), f)
    r_ = (np.arange(64) // 16) % 4
    p_ = np.arange(CP) % 4
    dcbv[:] = -0.5 * (4.0 * p_[None, :] + r_[:, None])
    red128v = np.zeros((128, 32), f)
    pidx = np.arange(128)
    red128v[pidx, (pidx // 64) * 16 + (pidx % 16)] = 1.0
    red64v = np.zeros((128, 64), f)
    red64v[pidx, pidx % 64] = 1.0
    red16v = np.zeros((64, 16), f)
    pidx = np.arange(64)
    red16v[pidx, pidx % 16] = 1.0
    return w1t, w2t, bdwv, bn1p, bnab, dcbv, red128v, red64v, red16v


def _im2col_conv1(xs):
    bl = xs.shape[0]
    sb, sc, sh, sw = xs.strides
    pat = np.lib.stride_tricks.as_strided(
        xs, (bl, 3, 14, 14, 5, 5), (sb, sc, 2 * sh, 2 * sw, sh, sw))
    return np.ascontiguousarray(
        pat.transpose(1, 4, 5, 0, 2, 3).reshape(75, bl * 196)).astype(np.float32)


def kernel(x, conv1_w, bn1_g, bn1_b, prim_w, bna_g, bna_b, bnp_g, bnp_b, W_ij):
    from concourse.bass_utils import run_bass_kernel_spmd
    x = np.asarray(x, np.float32)
    w1t, w2t, bdwv, bn1p, bnab, dcbv, red128v, red64v, red16v = _prep(
        np.asarray(conv1_w, np.float32), np.asarray(prim_w, np.float32),
        np.asarray(W_ij, np.float32), np.asarray(bn1_g, np.float32),
        np.asarray(bn1_b, np.float32), np.asarray(bna_g, np.float32),
        np.asarray(bna_b, np.float32), np.asarray(bnp_g, np.float32),
        np.asarray(bnp_b, np.float32))
    nc = _get_nc()
    in_maps = []
    for ci in range(NCORES):
        xs = x[ci * BL:(ci + 1) * BL]
        in_maps.append(dict(xcol=_im2col_conv1(xs), w1t=w1t, w2t=w2t, bdw=bdwv,
                            bn1p=bn1p, bnab=bnab, dcb=dcbv, red128=red128v,
                            red64=red64v, red16=red16v))
    res = run_bass_kernel_spmd(nc, in_maps, core_ids=list(range(NCORES)))
    return np.concatenate(
        [res.results[ci]["yh"] for ci in range(NCORES)], 0).astype(np.float32)
